# revision 28
# baseline (speedup 1.0000x reference)
"""LoRA-MoE Linear kernel for Trainium2, 8-core SPMD.

Strategy (token-parallel, bf16 compute). The PE is the bottleneck: a
board GPIO power throttle caps the clock at 13/16 x 2.4 GHz ~ 1.95 GHz
once all 8 cores stream matmuls, and the steady-state MM spacing is
exactly N/clk with zero overhead. So the kernel minimizes (a) PE
streaming cycles, (b) PE-idle startup, (c) post-last-MM drain tail:

  - 8192 tokens sharded across 8 cores (1024 each); host pre-lays-out
    operands so each DMA is contiguous per partition.
  - Startup is chip-HBM-bound (8 cores pull x shards + replicated
    weights simultaneously; ~160 GB/s/core effective). Pass 1 is
    therefore built to need only ~90 GB/s: it sweeps token-half th0
    through {tmp = A^T x, main oc0..4} consuming only the th0 halves
    of x, then sweeps th1. The gate hidden hT is deferred until x is
    fully resident. PSUM: 1 (tmp) + 5 (oc pos) + 2 (psc) = 8 banks.
  - Zero-matmul PE warmup from t~0.3us keeps the HAM clock-gate busy
    while the first x chunks are in flight (adds exact zeros to the
    tmp accumulation group).
  - hT is computed with 4-way PE column tiling: tile j (array
    col-group j, PSUM partitions 32j:32j+32) accumulates the
    k = j (mod 4) quarter of the contraction concurrently with the
    other three tiles -> ~4x fewer PE slots (verified ~3ns tile skew).
    A tiny S-matrix matmul (S[p,c] = [p%32==c]) recombines the four
    partials into token-major d[32,tok].
  - Routing stays expert-major [8,tok]: gpsimd partition_all_reduce for
    var / top-2 maxima; counts AllReduce (64B) overlaps oc6..oc11.
  - LoRA is fused into the main PSUM accumulation for oc>=FUSE_OC; the
    unfused oc<FUSE_OC LoRA blocks run interleaved between fused ocs
    (not at the end) so their loraT writes drain under compute and the
    kernel tail is just oc31's small split DMAs (issued on two DGE
    rings in parallel).
  - All outputs are written bf16 (outT, loraT); host upcasts + adds.
"""

import numpy as np
import ml_dtypes

import concourse.bacc as bacc
import concourse.bass as bass
import concourse.bass_isa as bass_isa
import concourse.mybir as mybir
import concourse.tile as tile
from concourse.bass_utils import run_bass_kernel_spmd

F32 = mybir.dt.float32
BF16 = mybir.dt.bfloat16
AX = mybir.AxisListType
ALU = mybir.AluOpType
ACT = mybir.ActivationFunctionType
RED = bass_isa.ReduceOp
BF = ml_dtypes.bfloat16

B, S, IN, OUT = 4, 2048, 4096, 4096
E, K, R = 8, 2, 16
CAP_FACTOR = 3.0
ALPHA = 1.0 / R
LN_EPS = 1e-5
N_CORES = 8
N_TOK = B * S               # 8192
TPC = N_TOK // N_CORES      # 1024 tokens per core
G4E = 4 * E                 # 32 gate hidden
ER = E * R                  # 128
KT = IN // 128              # 32 contraction tiles
OC = OUT // 128             # 32 output column blocks
NEG = -1.0e30
CAPACITY = float(int(CAP_FACTOR * N_TOK / E))  # 3072
TH = TPC // 512             # 2 token halves of 512
P1OC = 3                    # main ocs folded into pass 1
FUSE_OC = 13                # oc >= FUSE_OC get LoRA fused into main PSUM


def build_bass():
    nc = bacc.Bacc(
        "TRN2", target_bir_lowering=False, debug=False, num_devices=N_CORES
    )
    xp = nc.dram_tensor("xp", [128, KT * TPC], BF16, kind="ExternalInput")
    wp = nc.dram_tensor("wp", [OC, 128, KT * 128], BF16, kind="ExternalInput")
    ap_ = nc.dram_tensor("ap_", [128, KT * ER], BF16, kind="ExternalInput")
    g1p = nc.dram_tensor("g1p", [128, KT * G4E], BF16, kind="ExternalInput")
    bp = nc.dram_tensor("bp", [ER, OUT], BF16, kind="ExternalInput")
    g2p = nc.dram_tensor("g2p", [G4E, E], BF16, kind="ExternalInput")
    repp = nc.dram_tensor("repp", [E, ER], BF16, kind="ExternalInput")
    smat = nc.dram_tensor("smat", [128, G4E], BF16, kind="ExternalInput")
    gb1c = nc.dram_tensor("gb1c", [G4E, 1], F32, kind="ExternalInput")
    gamc = nc.dram_tensor("gamc", [G4E, 1], F32, kind="ExternalInput")
    betc = nc.dram_tensor("betc", [G4E, 1], F32, kind="ExternalInput")
    gb2c = nc.dram_tensor("gb2c", [E, 1], F32, kind="ExternalInput")
    outT = nc.dram_tensor("outT", [OUT, TPC], BF16, kind="ExternalOutput")
    loraT = nc.dram_tensor("loraT", [FUSE_OC * 128, TPC], BF16, kind="ExternalOutput")

    with tile.TileContext(nc) as tc:
        with (
            tc.tile_pool(name="big", bufs=1) as big,
            tc.tile_pool(name="rt", bufs=1) as rt,
            tc.tile_pool(name="wsl", bufs=6) as wsp,
            tc.tile_pool(name="outp", bufs=3) as op_,
            tc.tile_pool(name="ps_a", bufs=2, space="PSUM") as psa,
            tc.tile_pool(name="ps_m", bufs=6, space="PSUM") as psm,
            tc.tile_pool(name="dram", bufs=1, space="DRAM") as dp,
        ):
            # ---- PE warmup: zero-matmuls from t~0.3us ---------------------
            NWARM = 22
            warm_src = big.tile([128, 512], BF16)
            nc.vector.memset(warm_src, 0.0)
            # psa serves one [128,512] bank generation at a time:
            # tmp0 -> tmp1 -> hT0 -> hT1
            tmp0 = psa.tile([128, 512], F32, tag="acc", name="tmp0")
            for i in range(NWARM):
                nc.tensor.matmul(
                    tmp0, warm_src[:, 0:128], warm_src,
                    start=(i == 0), stop=False,
                )

            # ---- resident loads on two HWDGE rings ------------------------
            # sync ring: a + x th0 halves (in consumption order) + x th1;
            # scalar ring: pass-1 W slabs (k-chunked), then g1/b/late W.
            xT_sb = big.tile([128, KT, TPC], BF16)
            a_sb = big.tile([128, KT * ER], BF16)
            g1_sb = big.tile([128, KT * G4E], BF16)
            wslp1 = [
                wsp.tile([128, KT * 128], BF16, tag="wsl", name=f"wslp{i}")
                for i in range(P1OC)
            ]

            def x_load(k, th, eng=nc.sync):
                eng.dma_start(
                    xT_sb[:, k, th * 512 : (th + 1) * 512],
                    xp.ap()[:, k * TPC + th * 512 : k * TPC + (th + 1) * 512],
                )

            def a_load(k0, k1):
                nc.scalar.dma_start(
                    a_sb[:, k0 * ER : k1 * ER], ap_.ap()[:, k0 * ER : k1 * ER]
                )

            def g1_load(k0, k1):
                nc.scalar.dma_start(
                    g1_sb[:, k0 * G4E : k1 * G4E],
                    g1p.ap()[:, k0 * G4E : k1 * G4E],
                )

            def w_load(wsl, oc, k0, k1, eng=nc.scalar):
                eng.dma_start(
                    wsl[:, k0 * 128 : k1 * 128],
                    wp.ap()[oc][:, k0 * 128 : k1 * 128],
                )

            # Both rings carry ~6.2 MB during pass-1 (x evens + W oc0/oc2
            # on sync; x odds + W oc1 + a on scalar), chunks issued in
            # consumption order -- pass-1 consumes at ~174 GB/s total.
            def xf_load(k, eng):
                eng.dma_start(
                    xT_sb[:, k], xp.ap()[:, k * TPC : (k + 1) * TPC]
                )

            # sync ring
            x_load(0, 0)
            x_load(1, 0)
            x_load(2, 0)
            x_load(3, 0)
            w_load(wslp1[0], 0, 0, 4, eng=nc.sync)
            w_load(wslp1[2], 2, 0, 4, eng=nc.sync)
            xf_load(4, nc.sync)
            w_load(wslp1[0], 0, 4, 8, eng=nc.sync)
            xf_load(6, nc.sync)
            w_load(wslp1[2], 2, 4, 8, eng=nc.sync)
            xf_load(8, nc.sync)
            w_load(wslp1[0], 0, 8, 16, eng=nc.sync)
            xf_load(10, nc.sync)
            w_load(wslp1[2], 2, 8, 16, eng=nc.sync)
            xf_load(12, nc.sync)
            xf_load(14, nc.sync)
            w_load(wslp1[0], 0, 16, 24, eng=nc.sync)
            xf_load(16, nc.sync)
            w_load(wslp1[2], 2, 16, 24, eng=nc.sync)
            xf_load(18, nc.sync)
            xf_load(20, nc.sync)
            w_load(wslp1[0], 0, 24, 32, eng=nc.sync)
            xf_load(22, nc.sync)
            w_load(wslp1[2], 2, 24, 32, eng=nc.sync)
            for k in range(24, KT, 2):
                xf_load(k, nc.sync)
            # scalar ring
            a_load(0, 1)
            x_load(0, 1, eng=nc.scalar)
            x_load(1, 1, eng=nc.scalar)
            x_load(2, 1, eng=nc.scalar)
            x_load(3, 1, eng=nc.scalar)
            w_load(wslp1[1], 1, 0, 4)
            a_load(1, 8)
            w_load(wslp1[1], 1, 4, 8)
            xf_load(5, nc.scalar)
            xf_load(7, nc.scalar)
            w_load(wslp1[1], 1, 8, 16)
            xf_load(9, nc.scalar)
            a_load(8, 16)
            xf_load(11, nc.scalar)
            w_load(wslp1[1], 1, 16, 24)
            xf_load(13, nc.scalar)
            xf_load(15, nc.scalar)
            a_load(16, 24)
            w_load(wslp1[1], 1, 24, 32)
            xf_load(17, nc.scalar)
            xf_load(19, nc.scalar)
            a_load(24, 32)
            for k in range(21, KT, 2):
                xf_load(k, nc.scalar)
            g1_load(0, KT)
            b_sb = big.tile([ER, OUT], BF16)
            for hh in range(2):
                nc.scalar.dma_start(
                    b_sb[:, hh * 2048 : (hh + 1) * 2048],
                    bp.ap()[:, hh * 2048 : (hh + 1) * 2048],
                )
            g2_sb = big.tile([G4E, E], BF16)
            nc.scalar.dma_start(g2_sb, g2p.ap())
            repp_sb = big.tile([E, ER], BF16)
            nc.scalar.dma_start(repp_sb, repp.ap())
            smat_sb = big.tile([128, G4E], BF16)
            nc.scalar.dma_start(smat_sb, smat.ap())
            gb1c_sb = big.tile([G4E, 1], F32)
            nc.gpsimd.dma_start(gb1c_sb, gb1c.ap())
            gamc_sb = big.tile([G4E, 1], F32)
            nc.gpsimd.dma_start(gamc_sb, gamc.ap())
            betc_sb = big.tile([G4E, 1], F32)
            nc.gpsimd.dma_start(betc_sb, betc.ap())
            gb2c_sb = big.tile([E, 1], F32)
            nc.gpsimd.dma_start(gb2c_sb, gb2c.ap())
            eps_sb = big.tile([G4E, 1], F32)
            nc.vector.memset(eps_sb, LN_EPS)

            # ---- pass 1: two token-half sweeps of tmp + oc0..4 ------------
            tmp_sb = big.tile([128, TPC], F32)
            tmp_ps = [
                tmp0,
                psa.tile([128, 512], F32, tag="acc", name="tmp1"),
            ]
            posp1 = [
                [
                    psm.tile([128, 512], F32, tag="po", name=f"pop{j}_{t}")
                    for t in range(TH)
                ]
                for j in range(P1OC)
            ]
            for k in range(KT):
                first, last = k == 0, k == KT - 1
                for th in range(TH):
                    nc.tensor.matmul(
                        tmp_ps[th], a_sb[:, k * ER : (k + 1) * ER],
                        xT_sb[:, k, th * 512 : (th + 1) * 512],
                        # tmp0's group was opened by the warmup MMs
                        start=(first and th != 0), stop=last,
                    )
                for j in range(P1OC):
                    for th in range(TH):
                        nc.tensor.matmul(
                            posp1[j][th], wslp1[j][:, k * 128 : (k + 1) * 128],
                            xT_sb[:, k, th * 512 : (th + 1) * 512],
                            start=first, stop=last,
                        )
            for th in range(TH):
                sl = slice(th * 512, (th + 1) * 512)
                nc.vector.tensor_copy(tmp_sb[:, sl], tmp_ps[th])
                for j in range(P1OC):
                    osb = op_.tile([128, 512], BF16, tag="osb")
                    nc.scalar.activation(osb, posp1[j][th], ACT.Copy)
                    nc.sync.dma_start(
                        outT.ap()[j * 128 : (j + 1) * 128, sl], osb
                    )

            # ---- deferred gate hT: column-tiled bursts over resident x ----
            # tile j accumulates k = 4g+j; th sweeps are sequential so a
            # single PSUM bank generation is live at a time.
            hT_sb = big.tile([128, TH * 512], BF16)
            d_sb = big.tile([G4E, TPC], F32)
            for th in range(TH):
                sl = slice(th * 512, (th + 1) * 512)
                hT_ps = psa.tile([128, 512], F32, tag="acc", name=f"hT{th}")
                for g in range(KT // 4):
                    for j in range(4):
                        k = 4 * g + j
                        nc.tensor.matmul(
                            hT_ps[32 * j : 32 * (j + 1), :],
                            g1_sb[:, k * G4E : (k + 1) * G4E],
                            xT_sb[:, k, sl],
                            start=(g == 0), stop=(g == KT // 4 - 1),
                            tile_position=(0, 32 * j),
                        )
                nc.vector.tensor_copy(hT_sb[:, sl], hT_ps)
            d_ps = [
                psa.tile([G4E, 512], F32, tag="acc", name=f"d{t}")
                for t in range(TH)
            ]
            for th in range(TH):
                sl = slice(th * 512, (th + 1) * 512)
                nc.tensor.matmul(
                    d_ps[th], smat_sb, hT_sb[:, sl], start=True, stop=True
                )
                nc.vector.tensor_scalar(
                    out=d_sb[:, sl], in0=d_ps[th], scalar1=gb1c_sb,
                    scalar2=None, op0=ALU.add,
                )

            def main_oc(oc, fused, last=False):
                wsl = wsp.tile([128, KT * 128], BF16, tag="wsl")
                eng = nc.sync if oc % 2 == 0 else nc.scalar
                w_load(wsl, oc, 0, 16, eng=eng)
                w_load(wsl, oc, 16, 32, eng=eng)
                pos = [
                    psm.tile([128, 512], F32, tag="po", name=f"po{oc}_{t}")
                    for t in range(TH)
                ]
                for k in range(KT):
                    for th in range(TH):
                        nc.tensor.matmul(
                            pos[th], wsl[:, k * 128 : (k + 1) * 128],
                            xT_sb[:, k, th * 512 : (th + 1) * 512],
                            start=(k == 0),
                            stop=(not fused and k == KT - 1),
                        )
                for th in range(TH):
                    sl = slice(th * 512, (th + 1) * 512)
                    if fused:
                        nc.tensor.matmul(
                            pos[th], b_sb[:, oc * 128 : (oc + 1) * 128],
                            tw_bf[:, sl], start=False, stop=True,
                        )
                    osb = op_.tile([128, 512], BF16, tag="osb")
                    if last:
                        # end of kernel: copy in halves so the first DMA
                        # starts sooner; final DMAs go out on parallel
                        # engine rings
                        engs = [
                            (nc.sync, nc.scalar),
                            (nc.gpsimd, nc.sync),
                        ][th]
                        for hh in range(2):
                            csl = slice(hh * 256, (hh + 1) * 256)
                            nc.scalar.activation(
                                osb[:, csl], pos[th][:, csl], ACT.Copy
                            )
                            engs[hh].dma_start(
                                outT.ap()[
                                    oc * 128 : (oc + 1) * 128,
                                    th * 512 + hh * 256 : th * 512 + (hh + 1) * 256,
                                ],
                                osb[:, csl],
                            )
                    else:
                        nc.scalar.activation(osb, pos[th], ACT.Copy)
                        nc.sync.dma_start(
                            outT.ap()[oc * 128 : (oc + 1) * 128, sl], osb
                        )

            def lora_tail(oc):
                for th in range(TH):
                    sl = slice(th * 512, (th + 1) * 512)
                    lp = psm.tile([128, 512], F32, tag="po", name=f"lp{oc}_{th}")
                    nc.tensor.matmul(
                        lp, b_sb[:, oc * 128 : (oc + 1) * 128], tw_bf[:, sl],
                        start=True, stop=True,
                    )
                    lsb = op_.tile([128, 512], BF16, tag="lsb")
                    if th == 0:
                        nc.scalar.activation(lsb, lp, ACT.Copy)
                    else:
                        nc.vector.tensor_copy(lsb, lp)
                    nc.sync.dma_start(
                        loraT.ap()[oc * 128 : (oc + 1) * 128, sl], lsb
                    )

            tw_bf = big.tile([128, TPC], BF16)

            # ---- LayerNorm tail (vector/gpsimd/scalar only) ---------------
            sq = rt.tile([G4E, TPC], F32, tag="sq")
            nc.vector.tensor_tensor(out=sq, in0=d_sb, in1=d_sb, op=ALU.mult)
            varb = rt.tile([G4E, TPC], F32, tag="varb")
            nc.gpsimd.partition_all_reduce(varb, sq, channels=G4E, reduce_op=RED.add)
            rstd = rt.tile([G4E, TPC], F32, tag="rstd")
            nc.scalar.activation(
                rstd, varb, ACT.Sqrt, bias=eps_sb[:, :], scale=1.0 / G4E
            )
            nc.vector.reciprocal(rstd, rstd)
            nc.vector.tensor_tensor(out=d_sb, in0=d_sb, in1=rstd, op=ALU.mult)
            nc.vector.tensor_scalar(
                out=d_sb, in0=d_sb, scalar1=gamc_sb, scalar2=None, op0=ALU.mult
            )
            nc.vector.tensor_scalar(
                out=d_sb, in0=d_sb, scalar1=betc_sb, scalar2=None, op0=ALU.add
            )
            hn_bf = big.tile([G4E, TPC], BF16)
            nc.vector.tensor_scalar_max(hn_bf, d_sb, 0.0)

            # two main blocks cover the LN chain latency before the gates
            # matmuls enter the PE FIFO
            main_oc(P1OC, False)
            main_oc(P1OC + 1, False)

            gates = rt.tile([E, TPC], F32, tag="gates")
            for th in range(TH):
                sl = slice(th * 512, (th + 1) * 512)
                g_ps = psa.tile([E, 512], F32, tag="acc", name=f"g{th}")
                nc.tensor.matmul(g_ps, g2_sb, hn_bf[:, sl], start=True, stop=True)
                nc.vector.tensor_scalar(
                    out=gates[:, sl], in0=g_ps, scalar1=gb2c_sb,
                    scalar2=None, op0=ALU.add,
                )

            # ---- top-2 routing, expert-major ------------------------------
            v1 = rt.tile([E, TPC], F32, tag="v1")
            nc.gpsimd.partition_all_reduce(v1, gates, channels=E, reduce_op=RED.max)
            oh1 = rt.tile([E, TPC], F32, tag="oh1")
            nc.vector.tensor_tensor(out=oh1, in0=gates, in1=v1, op=ALU.is_ge)
            msk = rt.tile([E, TPC], F32, tag="msk")
            nc.vector.scalar_tensor_tensor(
                out=msk, in0=oh1, scalar=NEG, in1=gates, op0=ALU.mult, op1=ALU.add
            )
            v2 = rt.tile([E, TPC], F32, tag="v2")
            nc.gpsimd.partition_all_reduce(v2, msk, channels=E, reduce_op=RED.max)
            oh2 = rt.tile([E, TPC], F32, tag="oh2")
            nc.vector.tensor_tensor(out=oh2, in0=msk, in1=v2, op=ALU.is_ge)
            nc.vector.tensor_tensor(out=msk, in0=v1, in1=v2, op=ALU.subtract)
            s1 = rt.tile([E, TPC], F32, tag="s1")
            nc.scalar.activation(s1, msk, ACT.Sigmoid)
            u1 = rt.tile([E, TPC], F32, tag="u1")
            nc.vector.tensor_tensor(out=u1, in0=oh1, in1=s1, op=ALU.mult)
            u2 = rt.tile([E, TPC], F32, tag="u2")
            # u2 = oh2 * (1 - s1)
            nc.vector.scalar_tensor_tensor(
                out=u2, in0=s1, scalar=-1.0, in1=oh2, op0=ALU.mult, op1=ALU.add
            )
            nc.vector.tensor_tensor(out=u2, in0=u2, in1=oh2, op=ALU.mult)
            cnt = rt.tile([E, 2], F32, tag="cnt")
            nc.vector.tensor_reduce(out=cnt[:, 0:1], in_=oh1, axis=AX.X, op=ALU.add)
            nc.vector.tensor_reduce(out=cnt[:, 1:2], in_=oh2, axis=AX.X, op=ALU.add)
            cc_in = dp.tile([E, 2], F32)
            cc_out = dp.tile([E, 2], F32)
            nc.gpsimd.dma_start(cc_in, cnt)
            nc.gpsimd.collective_compute(
                "AllReduce",
                ALU.add,
                replica_groups=[list(range(N_CORES))],
                ins=[cc_in.opt()],
                outs=[cc_out.opt()],
            )
            cntg = rt.tile([E, 2], F32, tag="cntg")
            nc.gpsimd.dma_start(cntg, cc_out)

            # ---- unfused main blocks while the collective runs ------------
            for oc in range(P1OC + 2, FUSE_OC):
                main_oc(oc, False)

            # ---- post-collective combine (vector queue tail) --------------
            alw = rt.tile([E, 2], F32, tag="alw")
            nc.vector.tensor_scalar(
                out=alw, in0=cntg, scalar1=CAPACITY + 0.5, scalar2=None,
                op0=ALU.is_le,
            )
            q2 = rt.tile([E, TPC], F32, tag="q2")
            nc.vector.tensor_scalar(
                out=q2, in0=u2, scalar1=alw[:, 1:2], scalar2=None, op0=ALU.mult
            )
            w_bf = big.tile([E, TPC], BF16)
            nc.vector.scalar_tensor_tensor(
                out=w_bf, in0=u1, scalar=alw[:, 0:1], in1=q2,
                op0=ALU.mult, op1=ALU.add,
            )
            for th in range(TH):
                sl = slice(th * 512, (th + 1) * 512)
                wbr = psa.tile([128, 512], F32, tag="acc", name=f"wbr{th}")
                nc.tensor.matmul(wbr, repp_sb, w_bf[:, sl], start=True, stop=True)
                nc.vector.tensor_tensor(
                    out=tw_bf[:, sl], in0=tmp_sb[:, sl], in1=wbr, op=ALU.mult
                )

            # ---- fused main blocks, lora tails interleaved ----------------
            for oc in range(FUSE_OC, OC):
                main_oc(oc, fused=True, last=(oc == OC - 1))
                t = oc - FUSE_OC - 1  # tail t after fused oc FUSE_OC+1+t
                if 0 <= t < FUSE_OC:
                    lora_tail(t)
    return nc


_CACHE = {}


def _get_nc():
    if "nc" not in _CACHE:
        nc = build_bass()
        nc.finalize()
        _CACHE["nc"] = nc
    return _CACHE["nc"]


def prep_in_maps(inputs):
    x = np.asarray(inputs["x"], dtype=np.float32)
    weight = np.asarray(inputs["weight"], dtype=np.float32)
    xf = x.reshape(N_TOK, IN)
    # wp[oc, p, k*128+c] = weight[oc*128+c, k*128+p]
    wp = np.ascontiguousarray(
        weight.reshape(OC, 128, KT, 128).transpose(0, 3, 2, 1).reshape(OC, 128, KT * 128)
    ).astype(BF)
    a_cat = (
        np.asarray(inputs["lora_A"], np.float32).transpose(1, 0, 2).reshape(IN, ER)
        * ALPHA
    )
    ap_ = np.ascontiguousarray(
        a_cat.reshape(KT, 128, ER).transpose(1, 0, 2).reshape(128, KT * ER)
    ).astype(BF)
    # centered gate weights: LN mean subtraction folded into G1 and gb1
    g1T = np.asarray(inputs["gw1"], np.float32).T  # [IN, 32]
    g1T = g1T - g1T.mean(axis=1, keepdims=True)
    g1p = np.ascontiguousarray(
        g1T.reshape(KT, 128, G4E).transpose(1, 0, 2).reshape(128, KT * G4E)
    ).astype(BF)
    gb1 = np.asarray(inputs["gb1"], np.float32)
    gb1 = gb1 - gb1.mean()
    bp = np.asarray(inputs["lora_B"], np.float32).reshape(ER, OUT).astype(BF)
    g2p = np.ascontiguousarray(np.asarray(inputs["gw2"], np.float32).T).astype(BF)
    repm = np.zeros((E, ER), np.float32)
    for e in range(E):
        repm[e, e * R : (e + 1) * R] = 1.0
    repp = repm.astype(BF)
    # S-matrix: combine 4 column-group hT partials, S[p, c] = [p % 32 == c]
    sm = np.zeros((128, G4E), np.float32)
    for j in range(4):
        sm[32 * j : 32 * (j + 1)] = np.eye(G4E)
    smat = sm.astype(BF)
    gb1c = np.ascontiguousarray(gb1.reshape(G4E, 1))
    gamc = np.ascontiguousarray(
        np.asarray(inputs["ln_gamma"], np.float32).reshape(G4E, 1)
    )
    betc = np.ascontiguousarray(
        np.asarray(inputs["ln_beta"], np.float32).reshape(G4E, 1)
    )
    gb2c = np.ascontiguousarray(np.asarray(inputs["gb2"], np.float32).reshape(E, 1))

    shared = dict(
        wp=wp, ap_=ap_, g1p=g1p, bp=bp, g2p=g2p, repp=repp, smat=smat,
        gb1c=gb1c, gamc=gamc, betc=betc, gb2c=gb2c,
    )
    in_maps = []
    for c in range(N_CORES):
        xs = xf[c * TPC : (c + 1) * TPC]  # [TPC, IN]
        xpc = np.ascontiguousarray(
            xs.T.reshape(KT, 128, TPC).transpose(1, 0, 2).reshape(128, KT * TPC)
        ).astype(BF)
        in_maps.append(dict(xp=xpc, **shared))
    return in_maps


def gather(results):
    out = np.empty((N_TOK, OUT), np.float32)
    for c in range(N_CORES):
        tot = np.array(results[c]["outT"]).astype(np.float32)
        tot[: FUSE_OC * 128] += np.array(results[c]["loraT"]).astype(np.float32)
        out[c * TPC : (c + 1) * TPC] = tot.T
    return out.reshape(B, S, OUT)


def kernel(**inputs):
    in_maps = prep_in_maps(inputs)
    nc = _get_nc()
    res = run_bass_kernel_spmd(nc, in_maps, core_ids=list(range(N_CORES)))
    return gather(res.results)


# revision 32
# speedup vs baseline: 1.1032x; 1.1032x over previous
"""LoRA-MoE Linear kernel for Trainium2, 8-core SPMD.

Strategy (token-parallel, bf16 compute). The PE is the bottleneck: a
board GPIO power throttle caps the clock at 13/16 x 2.4 GHz ~ 1.95 GHz
once all 8 cores stream matmuls, and the steady-state MM spacing is
exactly N/clk with zero overhead. So the kernel minimizes (a) PE
streaming cycles, (b) PE-idle startup, (c) post-last-MM drain tail:

  - 8192 tokens sharded across 8 cores (1024 each); host pre-lays-out
    operands so each DMA is contiguous per partition.
  - Startup is chip-HBM-bound (8 cores pull x shards + replicated
    weights simultaneously; ~160 GB/s/core effective). Pass 1 is
    therefore built to need only ~90 GB/s: it sweeps token-half th0
    through {tmp = A^T x, main oc0..4} consuming only the th0 halves
    of x, then sweeps th1. The gate hidden hT is deferred until x is
    fully resident. PSUM: 1 (tmp) + 5 (oc pos) + 2 (psc) = 8 banks.
  - Zero-matmul PE warmup from t~0.3us keeps the HAM clock-gate busy
    while the first x chunks are in flight (adds exact zeros to the
    tmp accumulation group).
  - hT is computed with 4-way PE column tiling: tile j (array
    col-group j, PSUM partitions 32j:32j+32) accumulates the
    k = j (mod 4) quarter of the contraction concurrently with the
    other three tiles -> ~4x fewer PE slots (verified ~3ns tile skew).
    A tiny S-matrix matmul (S[p,c] = [p%32==c]) recombines the four
    partials into token-major d[32,tok].
  - Routing stays expert-major [8,tok]: gpsimd partition_all_reduce for
    var / top-2 maxima; counts AllReduce (64B) overlaps oc6..oc11.
  - LoRA is fused into the main PSUM accumulation for oc>=FUSE_OC; the
    unfused oc<FUSE_OC LoRA blocks run interleaved between fused ocs
    (not at the end) so their loraT writes drain under compute and the
    kernel tail is just oc31's small split DMAs (issued on two DGE
    rings in parallel).
  - All outputs are written bf16 (outT, loraT); host upcasts + adds.
"""

import numpy as np
import ml_dtypes

import concourse.bacc as bacc
import concourse.bass as bass
import concourse.bass_isa as bass_isa
import concourse.mybir as mybir
import concourse.tile as tile
from concourse.bass_utils import run_bass_kernel_spmd

F32 = mybir.dt.float32
BF16 = mybir.dt.bfloat16
AX = mybir.AxisListType
ALU = mybir.AluOpType
ACT = mybir.ActivationFunctionType
RED = bass_isa.ReduceOp
BF = ml_dtypes.bfloat16

B, S, IN, OUT = 4, 2048, 4096, 4096
E, K, R = 8, 2, 16
CAP_FACTOR = 3.0
ALPHA = 1.0 / R
LN_EPS = 1e-5
N_CORES = 8
N_TOK = B * S               # 8192
TPC = N_TOK // N_CORES      # 1024 tokens per core
G4E = 4 * E                 # 32 gate hidden
ER = E * R                  # 128
KT = IN // 128              # 32 contraction tiles
OC = OUT // 128             # 32 output column blocks
NEG = -1.0e30
CAPACITY = float(int(CAP_FACTOR * N_TOK / E))  # 3072
TH = TPC // 512             # 2 token halves of 512
P1OC = 3                    # main ocs folded into pass 1
FUSE_OC = 13                # oc >= FUSE_OC get LoRA fused into main PSUM


def build_bass():
    nc = bacc.Bacc(
        "TRN2", target_bir_lowering=False, debug=False, num_devices=N_CORES
    )
    xp = nc.dram_tensor("xp", [128, KT * TPC], BF16, kind="ExternalInput")
    wp = nc.dram_tensor("wp", [OC, 128, KT * 128], BF16, kind="ExternalInput")
    ap_ = nc.dram_tensor("ap_", [128, KT * ER], BF16, kind="ExternalInput")
    g1p = nc.dram_tensor("g1p", [128, KT * G4E], BF16, kind="ExternalInput")
    bp = nc.dram_tensor("bp", [ER, OUT], BF16, kind="ExternalInput")
    g2p = nc.dram_tensor("g2p", [G4E, E], BF16, kind="ExternalInput")
    repp = nc.dram_tensor("repp", [E, ER], BF16, kind="ExternalInput")
    smat = nc.dram_tensor("smat", [128, G4E], BF16, kind="ExternalInput")
    gb1c = nc.dram_tensor("gb1c", [G4E, 1], F32, kind="ExternalInput")
    gamc = nc.dram_tensor("gamc", [G4E, 1], F32, kind="ExternalInput")
    betc = nc.dram_tensor("betc", [G4E, 1], F32, kind="ExternalInput")
    gb2c = nc.dram_tensor("gb2c", [E, 1], F32, kind="ExternalInput")
    outT = nc.dram_tensor("outT", [OUT, TPC], BF16, kind="ExternalOutput")
    loraT = nc.dram_tensor("loraT", [FUSE_OC * 128, TPC], BF16, kind="ExternalOutput")

    with tile.TileContext(nc) as tc:
        with (
            tc.tile_pool(name="big", bufs=1) as big,
            tc.tile_pool(name="rt", bufs=1) as rt,
            tc.tile_pool(name="wsl", bufs=6) as wsp,
            tc.tile_pool(name="outp", bufs=2) as op_,
            tc.tile_pool(name="ps_a", bufs=2, space="PSUM") as psa,
            tc.tile_pool(name="ps_m", bufs=6, space="PSUM") as psm,
            tc.tile_pool(name="dram", bufs=1, space="DRAM") as dp,
        ):
            # ---- PE warmup: zero-matmuls from t~0.3us ---------------------
            NWARM = 12
            warm_src = big.tile([128, 512], F8)
            nc.vector.memset(warm_src, 0.0)
            # psa serves one [128,512] bank generation at a time:
            # tmp0 -> tmp1 -> hT0 -> hT1
            tmp0 = psa.tile([128, 512], F32, tag="acc", name="tmp0")
            for i in range(NWARM):
                nc.tensor.matmul(
                    tmp0, warm_src[:, 0:128], warm_src,
                    start=(i == 0), stop=False,
                )

            # ---- resident loads on two HWDGE rings ------------------------
            # sync ring: a + x th0 halves (in consumption order) + x th1;
            # scalar ring: pass-1 W slabs (k-chunked), then g1/b/late W.
            xT_sb = big.tile([128, KT, TPC], BF16)
            a_sb = big.tile([128, KT * ER], BF16)
            g1_sb = big.tile([128, KT * G4E], BF16)
            wslp1 = [
                wsp.tile([128, KT * 128], BF16, tag="wsl", name=f"wslp{i}")
                for i in range(P1OC)
            ]

            def x_load(k, th, eng=nc.sync):
                eng.dma_start(
                    xT_sb[:, k, th * 512 : (th + 1) * 512],
                    xp.ap()[:, k * TPC + th * 512 : k * TPC + (th + 1) * 512],
                )

            def a_load(k0, k1):
                nc.scalar.dma_start(
                    a_sb[:, k0 * ER : k1 * ER], ap_.ap()[:, k0 * ER : k1 * ER]
                )

            def g1_load(k0, k1):
                nc.scalar.dma_start(
                    g1_sb[:, k0 * G4E : k1 * G4E],
                    g1p.ap()[:, k0 * G4E : k1 * G4E],
                )

            def w_load(wsl, oc, k0, k1, eng=nc.scalar):
                eng.dma_start(
                    wsl[:, k0 * 128 : k1 * 128],
                    wp.ap()[oc][:, k0 * 128 : k1 * 128],
                )

            # Both rings carry ~6.2 MB during pass-1 (x evens + W oc0/oc2
            # on sync; x odds + W oc1 + a on scalar), chunks issued in
            # consumption order -- pass-1 consumes at ~174 GB/s total.
            def xf_load(k, eng):
                eng.dma_start(
                    xT_sb[:, k], xp.ap()[:, k * TPC : (k + 1) * TPC]
                )

            # sync ring
            x_load(0, 0)
            x_load(1, 0)
            x_load(2, 0)
            x_load(3, 0)
            w_load(wslp1[0], 0, 0, 4, eng=nc.sync)
            w_load(wslp1[2], 2, 0, 4, eng=nc.sync)
            xf_load(4, nc.sync)
            w_load(wslp1[0], 0, 4, 8, eng=nc.sync)
            xf_load(6, nc.sync)
            w_load(wslp1[2], 2, 4, 8, eng=nc.sync)
            xf_load(8, nc.sync)
            w_load(wslp1[0], 0, 8, 16, eng=nc.sync)
            xf_load(10, nc.sync)
            w_load(wslp1[2], 2, 8, 16, eng=nc.sync)
            xf_load(12, nc.sync)
            xf_load(14, nc.sync)
            w_load(wslp1[0], 0, 16, 24, eng=nc.sync)
            xf_load(16, nc.sync)
            w_load(wslp1[2], 2, 16, 24, eng=nc.sync)
            xf_load(18, nc.sync)
            xf_load(20, nc.sync)
            w_load(wslp1[0], 0, 24, 32, eng=nc.sync)
            xf_load(22, nc.sync)
            w_load(wslp1[2], 2, 24, 32, eng=nc.sync)
            for k in range(24, KT, 2):
                xf_load(k, nc.sync)
            # scalar ring
            a_load(0, 2)
            x_load(0, 1, eng=nc.scalar)
            x_load(1, 1, eng=nc.scalar)
            a_load(2, 8)
            x_load(2, 1, eng=nc.scalar)
            x_load(3, 1, eng=nc.scalar)
            w_load(wslp1[1], 1, 0, 4)
            w_load(wslp1[1], 1, 4, 8)
            xf_load(5, nc.scalar)
            xf_load(7, nc.scalar)
            w_load(wslp1[1], 1, 8, 16)
            xf_load(9, nc.scalar)
            a_load(8, 16)
            xf_load(11, nc.scalar)
            w_load(wslp1[1], 1, 16, 24)
            xf_load(13, nc.scalar)
            xf_load(15, nc.scalar)
            a_load(16, 24)
            w_load(wslp1[1], 1, 24, 32)
            xf_load(17, nc.scalar)
            xf_load(19, nc.scalar)
            a_load(24, 32)
            for k in range(21, KT, 2):
                xf_load(k, nc.scalar)
            g1_load(0, KT)
            b_sb = big.tile([ER, OUT], BF16)
            for hh in range(2):
                nc.scalar.dma_start(
                    b_sb[:, hh * 2048 : (hh + 1) * 2048],
                    bp.ap()[:, hh * 2048 : (hh + 1) * 2048],
                )
            g2_sb = big.tile([G4E, E], BF16)
            nc.scalar.dma_start(g2_sb, g2p.ap())
            repp_sb = big.tile([E, ER], BF16)
            nc.scalar.dma_start(repp_sb, repp.ap())
            smat_sb = big.tile([128, G4E], BF16)
            nc.scalar.dma_start(smat_sb, smat.ap())
            gb1c_sb = big.tile([G4E, 1], F32)
            nc.gpsimd.dma_start(gb1c_sb, gb1c.ap())
            gamc_sb = big.tile([G4E, 1], F32)
            nc.gpsimd.dma_start(gamc_sb, gamc.ap())
            betc_sb = big.tile([G4E, 1], F32)
            nc.gpsimd.dma_start(betc_sb, betc.ap())
            gb2c_sb = big.tile([E, 1], F32)
            nc.gpsimd.dma_start(gb2c_sb, gb2c.ap())
            eps_sb = big.tile([G4E, 1], F32)
            nc.vector.memset(eps_sb, LN_EPS)

            # ---- pass 1: two token-half sweeps of tmp + oc0..4 ------------
            tmp_sb = big.tile([128, TPC], F32)
            tmp_ps = [
                tmp0,
                psa.tile([128, 512], F32, tag="acc", name="tmp1"),
            ]
            posp1 = [
                [
                    psm.tile([128, 512], F32, tag="po", name=f"pop{j}_{t}")
                    for t in range(TH)
                ]
                for j in range(P1OC)
            ]
            for k in range(KT):
                first, last = k == 0, k == KT - 1
                for th in range(TH):
                    nc.tensor.matmul(
                        tmp_ps[th], a_sb[:, k * ER : (k + 1) * ER],
                        xT_sb[:, k, th * 512 : (th + 1) * 512],
                        # tmp0's group was opened by the warmup MMs
                        start=(first and th != 0), stop=last,
                    )
                for j in range(P1OC):
                    for th in range(TH):
                        nc.tensor.matmul(
                            posp1[j][th], wslp1[j][:, k * 128 : (k + 1) * 128],
                            xT_sb[:, k, th * 512 : (th + 1) * 512],
                            start=first, stop=last,
                        )
            for th in range(TH):
                sl = slice(th * 512, (th + 1) * 512)
                nc.vector.tensor_copy(tmp_sb[:, sl], tmp_ps[th])
                for j in range(P1OC):
                    osb = op_.tile([128, 512], BF16, tag="osb")
                    nc.scalar.activation(osb, posp1[j][th], ACT.Copy)
                    nc.sync.dma_start(
                        outT.ap()[j * 128 : (j + 1) * 128, sl], osb
                    )

            # ---- deferred gate hT: column-tiled bursts over resident x ----
            # tile j accumulates k = 4g+j; th sweeps are sequential so a
            # single PSUM bank generation is live at a time.
            hT_sb = big.tile([128, TH * 512], BF16)
            d_sb = big.tile([G4E, TPC], F32)
            for th in range(TH):
                sl = slice(th * 512, (th + 1) * 512)
                hT_ps = psa.tile([128, 512], F32, tag="acc", name=f"hT{th}")
                for g in range(KT // 4):
                    for j in range(4):
                        k = 4 * g + j
                        nc.tensor.matmul(
                            hT_ps[32 * j : 32 * (j + 1), :],
                            g1_sb[:, k * G4E : (k + 1) * G4E],
                            xT_sb[:, k, sl],
                            start=(g == 0), stop=(g == KT // 4 - 1),
                            tile_position=(0, 32 * j),
                        )
                nc.vector.tensor_copy(hT_sb[:, sl], hT_ps)
            d_ps = [
                psa.tile([G4E, 512], F32, tag="acc", name=f"d{t}")
                for t in range(TH)
            ]
            for th in range(TH):
                sl = slice(th * 512, (th + 1) * 512)
                nc.tensor.matmul(
                    d_ps[th], smat_sb, hT_sb[:, sl], start=True, stop=True
                )
                nc.vector.tensor_scalar(
                    out=d_sb[:, sl], in0=d_ps[th], scalar1=gb1c_sb,
                    scalar2=None, op0=ALU.add,
                )

            def main_oc(oc, fused, last=False):
                wsl = wsp.tile([128, KT * 128], BF16, tag="wsl")
                eng = nc.sync if oc % 2 == 0 else nc.scalar
                w_load(wsl, oc, 0, 16, eng=eng)
                w_load(wsl, oc, 16, 32, eng=eng)
                pos = [
                    psm.tile([128, 512], F32, tag="po", name=f"po{oc}_{t}")
                    for t in range(TH)
                ]
                for k in range(KT):
                    for th in range(TH):
                        nc.tensor.matmul(
                            pos[th], wsl[:, k * 128 : (k + 1) * 128],
                            xT_sb[:, k, th * 512 : (th + 1) * 512],
                            start=(k == 0),
                            stop=(not fused and k == KT - 1),
                        )
                for th in range(TH):
                    sl = slice(th * 512, (th + 1) * 512)
                    if fused:
                        nc.tensor.matmul(
                            pos[th], b_sb[:, oc * 128 : (oc + 1) * 128],
                            tw_bf[:, sl], start=False, stop=True,
                        )
                    osb = op_.tile([128, 512], BF16, tag="osb")
                    if last:
                        # end of kernel: copy in halves so the first DMA
                        # starts sooner; final DMAs go out on parallel
                        # engine rings
                        engs = [
                            (nc.sync, nc.scalar),
                            (nc.gpsimd, nc.sync),
                        ][th]
                        for hh in range(2):
                            csl = slice(hh * 256, (hh + 1) * 256)
                            nc.scalar.activation(
                                osb[:, csl], pos[th][:, csl], ACT.Copy
                            )
                            engs[hh].dma_start(
                                outT.ap()[
                                    oc * 128 : (oc + 1) * 128,
                                    th * 512 + hh * 256 : th * 512 + (hh + 1) * 256,
                                ],
                                osb[:, csl],
                            )
                    else:
                        nc.scalar.activation(osb, pos[th], ACT.Copy)
                        nc.sync.dma_start(
                            outT.ap()[oc * 128 : (oc + 1) * 128, sl], osb
                        )

            def lora_tail(oc):
                for th in range(TH):
                    sl = slice(th * 512, (th + 1) * 512)
                    lp = psm.tile([128, 512], F32, tag="po", name=f"lp{oc}_{th}")
                    nc.tensor.matmul(
                        lp, b_sb[:, oc * 128 : (oc + 1) * 128], tw_bf[:, sl],
                        start=True, stop=True,
                    )
                    lsb = op_.tile([128, 512], BF16, tag="osb", name="lsb")
                    if th == 0:
                        nc.scalar.activation(lsb, lp, ACT.Copy)
                    else:
                        nc.vector.tensor_copy(lsb, lp)
                    nc.sync.dma_start(
                        loraT.ap()[oc * 128 : (oc + 1) * 128, sl], lsb
                    )

            tw_bf = big.tile([128, TPC], BF16)

            # ---- LayerNorm tail (vector/gpsimd/scalar only) ---------------
            sq = rt.tile([G4E, TPC], F32, tag="lnt", name="sq")
            nc.vector.tensor_tensor(out=sq, in0=d_sb, in1=d_sb, op=ALU.mult)
            varb = rt.tile([G4E, TPC], F32, tag="varb")
            nc.gpsimd.partition_all_reduce(varb, sq, channels=G4E, reduce_op=RED.add)
            rstd = rt.tile([G4E, TPC], F32, tag="lnt", name="rstd")
            nc.scalar.activation(
                rstd, varb, ACT.Sqrt, bias=eps_sb[:, :], scale=1.0 / G4E
            )
            nc.vector.reciprocal(rstd, rstd)
            nc.vector.tensor_tensor(out=d_sb, in0=d_sb, in1=rstd, op=ALU.mult)
            nc.vector.tensor_scalar(
                out=d_sb, in0=d_sb, scalar1=gamc_sb, scalar2=None, op0=ALU.mult
            )
            nc.vector.tensor_scalar(
                out=d_sb, in0=d_sb, scalar1=betc_sb, scalar2=None, op0=ALU.add
            )
            hn_bf = big.tile([G4E, TPC], BF16)
            nc.vector.tensor_scalar_max(hn_bf, d_sb, 0.0)

            # two main blocks cover the LN chain latency before the gates
            # matmuls enter the PE FIFO
            main_oc(P1OC, False)
            main_oc(P1OC + 1, False)

            gates = rt.tile([E, TPC], F32, tag="gates")
            for th in range(TH):
                sl = slice(th * 512, (th + 1) * 512)
                g_ps = psa.tile([E, 512], F32, tag="acc", name=f"g{th}")
                nc.tensor.matmul(g_ps, g2_sb, hn_bf[:, sl], start=True, stop=True)
                nc.vector.tensor_scalar(
                    out=gates[:, sl], in0=g_ps, scalar1=gb2c_sb,
                    scalar2=None, op0=ALU.add,
                )

            # ---- top-2 routing, expert-major ------------------------------
            v1 = rt.tile([E, TPC], F32, tag="v1")
            nc.gpsimd.partition_all_reduce(v1, gates, channels=E, reduce_op=RED.max)
            oh1 = rt.tile([E, TPC], F32, tag="oh1")
            nc.vector.tensor_tensor(out=oh1, in0=gates, in1=v1, op=ALU.is_ge)
            msk = rt.tile([E, TPC], F32, tag="msk")
            nc.vector.scalar_tensor_tensor(
                out=msk, in0=oh1, scalar=NEG, in1=gates, op0=ALU.mult, op1=ALU.add
            )
            v2 = rt.tile([E, TPC], F32, tag="v2")
            nc.gpsimd.partition_all_reduce(v2, msk, channels=E, reduce_op=RED.max)
            oh2 = rt.tile([E, TPC], F32, tag="oh2")
            nc.vector.tensor_tensor(out=oh2, in0=msk, in1=v2, op=ALU.is_ge)
            nc.vector.tensor_tensor(out=msk, in0=v1, in1=v2, op=ALU.subtract)
            s1 = rt.tile([E, TPC], F32, tag="s1")
            nc.scalar.activation(s1, msk, ACT.Sigmoid)
            u1 = rt.tile([E, TPC], F32, tag="u1")
            nc.vector.tensor_tensor(out=u1, in0=oh1, in1=s1, op=ALU.mult)
            u2 = rt.tile([E, TPC], F32, tag="u2")
            # u2 = oh2 * (1 - s1)
            nc.vector.scalar_tensor_tensor(
                out=u2, in0=s1, scalar=-1.0, in1=oh2, op0=ALU.mult, op1=ALU.add
            )
            nc.vector.tensor_tensor(out=u2, in0=u2, in1=oh2, op=ALU.mult)
            cnt = rt.tile([E, 2], F32, tag="cnt")
            nc.vector.tensor_reduce(out=cnt[:, 0:1], in_=oh1, axis=AX.X, op=ALU.add)
            nc.vector.tensor_reduce(out=cnt[:, 1:2], in_=oh2, axis=AX.X, op=ALU.add)
            cc_in = dp.tile([E, 2], F32)
            cc_out = dp.tile([E, 2], F32)
            nc.gpsimd.dma_start(cc_in, cnt)
            nc.gpsimd.collective_compute(
                "AllReduce",
                ALU.add,
                replica_groups=[list(range(N_CORES))],
                ins=[cc_in.opt()],
                outs=[cc_out.opt()],
            )
            cntg = rt.tile([E, 2], F32, tag="cntg")
            nc.gpsimd.dma_start(cntg, cc_out)

            # ---- unfused main blocks while the collective runs ------------
            for oc in range(P1OC + 2, FUSE_OC):
                main_oc(oc, False)

            # ---- post-collective combine (vector queue tail) --------------
            alw = rt.tile([E, 2], F32, tag="alw")
            nc.vector.tensor_scalar(
                out=alw, in0=cntg, scalar1=CAPACITY + 0.5, scalar2=None,
                op0=ALU.is_le,
            )
            q2 = rt.tile([E, TPC], F32, tag="q2")
            nc.vector.tensor_scalar(
                out=q2, in0=u2, scalar1=alw[:, 1:2], scalar2=None, op0=ALU.mult
            )
            w_bf = big.tile([E, TPC], BF16)
            nc.vector.scalar_tensor_tensor(
                out=w_bf, in0=u1, scalar=alw[:, 0:1], in1=q2,
                op0=ALU.mult, op1=ALU.add,
            )
            for th in range(TH):
                sl = slice(th * 512, (th + 1) * 512)
                wbr = psa.tile([128, 512], F32, tag="acc", name=f"wbr{th}")
                nc.tensor.matmul(wbr, repp_sb, w_bf[:, sl], start=True, stop=True)
                nc.vector.tensor_tensor(
                    out=tw_bf[:, sl], in0=tmp_sb[:, sl], in1=wbr, op=ALU.mult
                )

            # ---- fused main blocks, lora tails interleaved ----------------
            for oc in range(FUSE_OC, OC):
                main_oc(oc, fused=True, last=(oc == OC - 1))
                t = oc - FUSE_OC - 1  # tail t after fused oc FUSE_OC+1+t
                if 0 <= t < FUSE_OC:
                    lora_tail(t)
    return nc


_CACHE = {}


def _get_nc():
    if "nc" not in _CACHE:
        nc = build_bass()
        nc.finalize()
        _CACHE["nc"] = nc
    return _CACHE["nc"]


def prep_in_maps(inputs):
    x = np.asarray(inputs["x"], dtype=np.float32)
    weight = np.asarray(inputs["weight"], dtype=np.float32)
    xf = x.reshape(N_TOK, IN)
    # wp[oc, p, k*128+c] = weight[oc*128+c, k*128+p]
    wp = np.ascontiguousarray(
        weight.reshape(OC, 128, KT, 128).transpose(0, 3, 2, 1).reshape(OC, 128, KT * 128)
    ).astype(BF)
    a_cat = (
        np.asarray(inputs["lora_A"], np.float32).transpose(1, 0, 2).reshape(IN, ER)
        * ALPHA
    )
    ap_ = np.ascontiguousarray(
        a_cat.reshape(KT, 128, ER).transpose(1, 0, 2).reshape(128, KT * ER)
    ).astype(BF)
    # centered gate weights: LN mean subtraction folded into G1 and gb1
    g1T = np.asarray(inputs["gw1"], np.float32).T  # [IN, 32]
    g1T = g1T - g1T.mean(axis=1, keepdims=True)
    g1p = np.ascontiguousarray(
        g1T.reshape(KT, 128, G4E).transpose(1, 0, 2).reshape(128, KT * G4E)
    ).astype(BF)
    gb1 = np.asarray(inputs["gb1"], np.float32)
    gb1 = gb1 - gb1.mean()
    bp = np.asarray(inputs["lora_B"], np.float32).reshape(ER, OUT).astype(BF)
    g2p = np.ascontiguousarray(np.asarray(inputs["gw2"], np.float32).T).astype(BF)
    repm = np.zeros((E, ER), np.float32)
    for e in range(E):
        repm[e, e * R : (e + 1) * R] = 1.0
    repp = repm.astype(BF)
    # S-matrix: combine 4 column-group hT partials, S[p, c] = [p % 32 == c]
    sm = np.zeros((128, G4E), np.float32)
    for j in range(4):
        sm[32 * j : 32 * (j + 1)] = np.eye(G4E)
    smat = sm.astype(BF)
    gb1c = np.ascontiguousarray(gb1.reshape(G4E, 1))
    gamc = np.ascontiguousarray(
        np.asarray(inputs["ln_gamma"], np.float32).reshape(G4E, 1)
    )
    betc = np.ascontiguousarray(
        np.asarray(inputs["ln_beta"], np.float32).reshape(G4E, 1)
    )
    gb2c = np.ascontiguousarray(np.asarray(inputs["gb2"], np.float32).reshape(E, 1))

    shared = dict(
        wp=wp, ap_=ap_, g1p=g1p, bp=bp, g2p=g2p, repp=repp, smat=smat,
        gb1c=gb1c, gamc=gamc, betc=betc, gb2c=gb2c,
    )
    in_maps = []
    for c in range(N_CORES):
        xs = xf[c * TPC : (c + 1) * TPC]  # [TPC, IN]
        xpc = np.ascontiguousarray(
            xs.T.reshape(KT, 128, TPC).transpose(1, 0, 2).reshape(128, KT * TPC)
        ).astype(BF)
        in_maps.append(dict(xp=xpc, **shared))
    return in_maps


def gather(results):
    out = np.empty((N_TOK, OUT), np.float32)
    for c in range(N_CORES):
        tot = np.array(results[c]["outT"]).astype(np.float32)
        tot[: FUSE_OC * 128] += np.array(results[c]["loraT"]).astype(np.float32)
        out[c * TPC : (c + 1) * TPC] = tot.T
    return out.reshape(B, S, OUT)


def kernel(**inputs):
    in_maps = prep_in_maps(inputs)
    nc = _get_nc()
    res = run_bass_kernel_spmd(nc, in_maps, core_ids=list(range(N_CORES)))
    return gather(res.results)
